# revision 10
# baseline (speedup 1.0000x reference)
"""Bass/Trainium2 kernel for nn_CMOS_60181081752266.

Computes, for each of 10 classes, sum(|patch|) where patch is a static
273x273 crop of the 8192x8192 input X. Only ~3MB of X is ever needed, so
the host slices the 10 patches out of X and repacks them so that class
boundaries fall at *static column positions*, identical on every core
(SPMD-friendly):

  core k input x [128, 729] f32:
    cols [0, 583)   = class k          (74529 elems padded to 128x583)
    cols [583, 729) = quarter (k%4) of class 8 + k//4  (padded 128x584,
                      cols [146q, 146q+146))

Per core:
  - x is DMA'd in 4 column chunks, 2 on each HWDGE queue (sync + scalar)
    to hide HBM latency
  - DVE reduces each class-pure column range with apply_absolute_value
    into separate columns of pad [128, 8] (5 partials: 4 class-A chunk
    partials + 1 class-B)
  - PE matmul ones[128,1].T @ pad[:,0:5] collapses partitions -> psum [1,5]
  - ACT copies psum to SBUF and issues a single-descriptor 20B DMA of
    y [1, 5]

Host sums the handful of partials per class across cores.
"""

import numpy as np

import concourse.bass as bass
import concourse.mybir as mybir
from concourse.bass_utils import run_bass_kernel_spmd

CLASSES = 10
FRAME_S = 273          # 8192 // (10*3)
N_CORES = 8
P = 128                # SBUF partitions per core
FA = 583               # class-A columns: 128*583 = 74624 >= 273*273
FB = 146               # class-B quarter columns: 4*146 = 584, 128*584 >= 74529
F = FA + FB            # 729
YCOLS = 5
# DMA chunk boundaries (4 chunks over 2 queues)
CUTS = [0, 183, 366, 549, F]
# reduce ranges (class-pure) -> pad column, gated by which DMA chunk
# reduce j covers [RCUTS[j], RCUTS[j+1])
RCUTS = [0, 183, 366, 549, FA, F]


def _starts():
    # cols = floor(sqrt(10)) + 1 = 4, cell = 8192 // 4 = 2048
    # xs = (i%4)*2048 + 1024 - 136, ys = (i//4)*2048 + 1024 - 136
    return [(888 + 2048 * (i % 4), 888 + 2048 * (i // 4)) for i in range(CLASSES)]


def _build_bass():
    f32 = mybir.dt.float32
    nc = bass.Bass()
    x = nc.dram_tensor("x", [P, F], f32, kind="ExternalInput")
    y = nc.dram_tensor("y", [1, YCOLS], f32, kind="ExternalOutput")
    with (
        nc.sbuf_tensor("t", [P, F], f32) as t,
        nc.sbuf_tensor("pad", [P, 8], f32) as pad,
        nc.sbuf_tensor("ones", [P, 1], f32) as ones,
        nc.sbuf_tensor("zt", [1, YCOLS], f32) as zt,
        nc.psum_tensor("ps", [P, 8], f32) as ps,
        nc.semaphore("c0sem") as c0sem,
        nc.semaphore("c1sem") as c1sem,
        nc.semaphore("c2sem") as c2sem,
        nc.semaphore("c3sem") as c3sem,
        nc.semaphore("ysem") as ysem,
        nc.semaphore("gsem") as gsem,
        nc.semaphore("vsem") as vsem,
        nc.semaphore("psem") as psem,
        nc.Block() as block,
    ):

        @block.gpsimd
        def _(gpsimd):
            gpsimd.memset(ones[:], 1.0).then_inc(gsem, 1)

        @block.sync
        def _(sync):
            sync.dma_start(t[:, CUTS[0] : CUTS[1]], x[:, CUTS[0] : CUTS[1]]).then_inc(
                c0sem, 16
            )
            sync.dma_start(t[:, CUTS[2] : CUTS[3]], x[:, CUTS[2] : CUTS[3]]).then_inc(
                c2sem, 16
            )

        @block.scalar
        def _(scalar):
            scalar.dma_start(
                t[:, CUTS[1] : CUTS[2]], x[:, CUTS[1] : CUTS[2]]
            ).then_inc(c1sem, 16)
            scalar.dma_start(
                t[:, CUTS[3] : CUTS[4]], x[:, CUTS[3] : CUTS[4]]
            ).then_inc(c3sem, 16)
            scalar.wait_ge(psem, 1)
            scalar.copy(zt[:], ps[0:1, 0:YCOLS])
            scalar.drain()
            scalar.dma_start(y[:], zt[:]).then_inc(ysem, 16)
            scalar.wait_ge(ysem, 16)

        @block.vector
        def _(vector):
            sems = [c0sem, c1sem, c2sem, c3sem, c3sem]
            for j in range(5):
                vector.wait_ge(sems[j], 16)
                ins = vector.tensor_reduce(
                    pad[:, j : j + 1],
                    t[:, RCUTS[j] : RCUTS[j + 1]],
                    axis=mybir.AxisListType.X,
                    op=mybir.AluOpType.add,
                    apply_absolute_value=True,
                )
            ins.then_inc(vsem, 1)

        @block.tensor
        def _(tensor):
            tensor.wait_ge(gsem, 1)
            tensor.wait_ge(vsem, 1)
            nc.tensor.matmul(
                ps[0:1, 0:YCOLS], ones[:], pad[:, 0:YCOLS]
            ).then_inc(psem, 1)

    return nc


def _prep_in_maps(X: np.ndarray):
    X = np.ascontiguousarray(X, dtype=np.float32)
    starts = _starts()
    flats = []
    for c, (xs, ys) in enumerate(starts):
        flats.append(X[xs : xs + FRAME_S, ys : ys + FRAME_S].reshape(-1))

    in_maps = []
    for k in range(N_CORES):
        xk = np.zeros((P, F), dtype=np.float32)
        # class A = class k
        bufA = np.zeros(P * FA, dtype=np.float32)
        bufA[: FRAME_S * FRAME_S] = flats[k]
        xk[:, :FA] = bufA.reshape(P, FA)
        # class B = quarter (k%4) of class 8 + k//4
        cb = 8 + k // 4
        q = k % 4
        bufB = np.zeros(P * 4 * FB, dtype=np.float32)
        bufB[: FRAME_S * FRAME_S] = flats[cb]
        xk[:, FA:] = bufB.reshape(P, 4 * FB)[:, FB * q : FB * (q + 1)]
        in_maps.append({"x": xk})
    return in_maps


_NC = None


def kernel(X: np.ndarray) -> np.ndarray:
    global _NC
    if _NC is None:
        _NC = _build_bass()
    in_maps = _prep_in_maps(X)
    res = run_bass_kernel_spmd(_NC, in_maps, core_ids=list(range(N_CORES)))
    out = np.zeros(CLASSES, dtype=np.float32)
    for k in range(N_CORES):
        yk = res.results[k]["y"].reshape(-1)
        out[k] += yk[0:4].sum(dtype=np.float32)
        out[8 + k // 4] += yk[4]
    return out.astype(np.float32)


# revision 14
# speedup vs baseline: 1.1071x; 1.1071x over previous
"""Bass/Trainium2 kernel for nn_CMOS_60181081752266.

Computes, for each of 10 classes, sum(|patch|) where patch is a static
273x273 crop of the 8192x8192 input X. Only ~3MB of X is ever needed, so
the host slices the 10 patches out of X and repacks them so that class
boundaries fall at *static column positions*, identical on every core
(SPMD-friendly):

  core k input x [128, 729] f32:
    cols [0, 583)   = class k          (74529 elems padded to 128x583)
    cols [583, 729) = quarter (k%4) of class 8 + k//4  (padded 128x584,
                      cols [146q, 146q+146))

Per core:
  - x is DMA'd in 4 column chunks, 2 on each HWDGE queue (sync + scalar)
    to hide HBM latency
  - DVE reduces each class-pure column range with apply_absolute_value
    into separate columns of pad [128, 8] (5 partials: 4 class-A chunk
    partials + 1 class-B)
  - PE matmul ones[128,1].T @ pad[:,0:5] collapses partitions -> psum [1,5]
  - ACT copies psum to SBUF and issues a single-descriptor 20B DMA of
    y [1, 5]

Host sums the handful of partials per class across cores.
"""

import numpy as np

import concourse.bass as bass
import concourse.mybir as mybir
from concourse.bass_utils import run_bass_kernel_spmd

CLASSES = 10
FRAME_S = 273          # 8192 // (10*3)
N_CORES = 8
P = 128                # SBUF partitions per core
FA = 583               # class-A columns: 128*583 = 74624 >= 273*273
FB = 146               # class-B quarter columns: 4*146 = 584, 128*584 >= 74529
F = FA + FB            # 729
YCOLS = 5
# DMA chunk boundaries (4 chunks over 2 queues)
CUTS = [0, 183, 366, 549, F]
# reduce ranges (class-pure) -> pad column, gated by which DMA chunk
# reduce j covers [RCUTS[j], RCUTS[j+1])
RCUTS = [0, 183, 366, 549, FA, F]


def _starts():
    # cols = floor(sqrt(10)) + 1 = 4, cell = 8192 // 4 = 2048
    # xs = (i%4)*2048 + 1024 - 136, ys = (i//4)*2048 + 1024 - 136
    return [(888 + 2048 * (i % 4), 888 + 2048 * (i // 4)) for i in range(CLASSES)]


def _build_bass():
    f32 = mybir.dt.float32
    nc = bass.Bass()
    x = nc.dram_tensor("x", [P, F], f32, kind="ExternalInput")
    y = nc.dram_tensor("y", [1, YCOLS], f32, kind="ExternalOutput")
    with (
        nc.sbuf_tensor("t", [P, F], f32) as t,
        nc.sbuf_tensor("pad", [P, 8], f32) as pad,
        nc.sbuf_tensor("ones", [P, 1], f32) as ones,
        nc.sbuf_tensor("zt", [1, YCOLS], f32) as zt,
        nc.psum_tensor("ps", [P, 8], f32) as ps,
        nc.semaphore("c0sem") as c0sem,
        nc.semaphore("c1sem") as c1sem,
        nc.semaphore("c2sem") as c2sem,
        nc.semaphore("c3sem") as c3sem,
        nc.semaphore("ysem") as ysem,
        nc.semaphore("gsem") as gsem,
        nc.semaphore("vsem") as vsem,
        nc.semaphore("psem") as psem,
        nc.semaphore("zsem") as zsem,
        nc.Block() as block,
    ):

        @block.gpsimd
        def _(gpsimd):
            gpsimd.memset(ones[:], 1.0).then_inc(gsem, 1)

        @block.sync
        def _(sync):
            sync.dma_start(t[:, CUTS[0] : CUTS[1]], x[:, CUTS[0] : CUTS[1]]).then_inc(
                c0sem, 16
            )
            sync.dma_start(t[:, CUTS[2] : CUTS[3]], x[:, CUTS[2] : CUTS[3]]).then_inc(
                c2sem, 16
            )
            sync.wait_ge(zsem, 1)
            sync.dma_start(y[:], zt[:]).then_inc(ysem, 16)
            sync.wait_ge(ysem, 16)

        @block.scalar
        def _(scalar):
            scalar.dma_start(
                t[:, CUTS[1] : CUTS[2]], x[:, CUTS[1] : CUTS[2]]
            ).then_inc(c1sem, 16)
            scalar.dma_start(
                t[:, CUTS[3] : CUTS[4]], x[:, CUTS[3] : CUTS[4]]
            ).then_inc(c3sem, 16)

        @block.vector
        def _(vector):
            sems = [c0sem, c1sem, c2sem, c3sem, c3sem]
            for j in range(5):
                vector.wait_ge(sems[j], 16)
                ins = vector.tensor_reduce(
                    pad[:, j : j + 1],
                    t[:, RCUTS[j] : RCUTS[j + 1]],
                    axis=mybir.AxisListType.X,
                    op=mybir.AluOpType.add,
                    apply_absolute_value=True,
                )
            ins.then_inc(vsem, 1)
            vector.wait_ge(psem, 1)
            vector.tensor_copy(zt[:], ps[0:1, 0:YCOLS]).then_inc(zsem, 1)

        @block.tensor
        def _(tensor):
            tensor.wait_ge(gsem, 1)
            tensor.wait_ge(vsem, 1)
            nc.tensor.matmul(
                ps[0:1, 0:YCOLS], ones[:], pad[:, 0:YCOLS]
            ).then_inc(psem, 1)

    return nc


def _prep_in_maps(X: np.ndarray):
    X = np.ascontiguousarray(X, dtype=np.float32)
    starts = _starts()
    flats = []
    for c, (xs, ys) in enumerate(starts):
        flats.append(X[xs : xs + FRAME_S, ys : ys + FRAME_S].reshape(-1))

    in_maps = []
    for k in range(N_CORES):
        xk = np.zeros((P, F), dtype=np.float32)
        # class A = class k
        bufA = np.zeros(P * FA, dtype=np.float32)
        bufA[: FRAME_S * FRAME_S] = flats[k]
        xk[:, :FA] = bufA.reshape(P, FA)
        # class B = quarter (k%4) of class 8 + k//4
        cb = 8 + k // 4
        q = k % 4
        bufB = np.zeros(P * 4 * FB, dtype=np.float32)
        bufB[: FRAME_S * FRAME_S] = flats[cb]
        xk[:, FA:] = bufB.reshape(P, 4 * FB)[:, FB * q : FB * (q + 1)]
        in_maps.append({"x": xk})
    return in_maps


_NC = None


def kernel(X: np.ndarray) -> np.ndarray:
    global _NC
    if _NC is None:
        _NC = _build_bass()
    in_maps = _prep_in_maps(X)
    res = run_bass_kernel_spmd(_NC, in_maps, core_ids=list(range(N_CORES)))
    out = np.zeros(CLASSES, dtype=np.float32)
    for k in range(N_CORES):
        yk = res.results[k]["y"].reshape(-1)
        out[k] += yk[0:4].sum(dtype=np.float32)
        out[8 + k // 4] += yk[4]
    return out.astype(np.float32)


# revision 15
# speedup vs baseline: 1.2384x; 1.1186x over previous
"""Bass/Trainium2 kernel for nn_CMOS_60181081752266.

Computes, for each of 10 classes, sum(|patch|) where patch is a static
273x273 crop of the 8192x8192 input X. Only ~3MB of X is ever needed, so
the host slices the 10 patches out of X and repacks them so that class
boundaries fall at *static column positions*, identical on every core
(SPMD-friendly):

  core k logical input [128, 729] f32:
    cols [0, 583)   = class k          (74529 elems padded to 128x583)
    cols [583, 729) = quarter (k%4) of class 8 + k//4  (padded 128x584,
                      cols [146q, 146q+146))
  shipped chunk-major (each DMA chunk contiguous in DRAM for HBM page
  locality): flat [93312] = [128x180 | 128x220 | 128x183 | 128x146]

Per core:
  - 4 column chunks DMA'd, 2 on each HWDGE queue (sync + scalar)
  - DVE reduces each class-pure chunk with apply_absolute_value into
    separate columns of pad [128, 8] (3 class-A partials + 1 class-B)
  - PE matmul ones[128,1].T @ pad[:,0:4] collapses partitions -> psum [1,4]
  - DVE copies psum to SBUF; sync issues a single-descriptor 16B DMA of
    y [1, 4]

Host sums the handful of partials per class across cores.
"""

import numpy as np

import concourse.bass as bass
import concourse.mybir as mybir
from concourse.bass_utils import run_bass_kernel_spmd

CLASSES = 10
FRAME_S = 273          # 8192 // (10*3)
N_CORES = 8
P = 128                # SBUF partitions per core
FA = 583               # class-A columns: 128*583 = 74624 >= 273*273
FB = 146               # class-B quarter columns: 4*146 = 584, 128*584 >= 74529
F = FA + FB            # 729
YCOLS = 4
CUTS = [0, 180, 400, 583, F]   # chunk/reduce boundaries (class-pure)


def _starts():
    # cols = floor(sqrt(10)) + 1 = 4, cell = 8192 // 4 = 2048
    # xs = (i%4)*2048 + 1024 - 136, ys = (i//4)*2048 + 1024 - 136
    return [(888 + 2048 * (i % 4), 888 + 2048 * (i // 4)) for i in range(CLASSES)]


def _build_bass():
    f32 = mybir.dt.float32
    nc = bass.Bass()
    x = nc.dram_tensor("x", [P * F], f32, kind="ExternalInput")
    y = nc.dram_tensor("y", [1, YCOLS], f32, kind="ExternalOutput")
    with (
        nc.sbuf_tensor("t", [P, F], f32) as t,
        nc.sbuf_tensor("pad", [P, 8], f32) as pad,
        nc.sbuf_tensor("ones", [P, 1], f32) as ones,
        nc.sbuf_tensor("zt", [1, YCOLS], f32) as zt,
        nc.psum_tensor("ps", [P, 8], f32) as ps,
        nc.semaphore("c0sem") as c0sem,
        nc.semaphore("c1sem") as c1sem,
        nc.semaphore("c2sem") as c2sem,
        nc.semaphore("c3sem") as c3sem,
        nc.semaphore("ysem") as ysem,
        nc.semaphore("gsem") as gsem,
        nc.semaphore("vsem") as vsem,
        nc.semaphore("psem") as psem,
        nc.semaphore("zsem") as zsem,
    ):
        csems = [c0sem, c1sem, c2sem, c3sem]

        def chunk_src(j):
            off = P * CUTS[j]
            w = CUTS[j + 1] - CUTS[j]
            return x[off : off + P * w].rearrange("(p c) -> p c", p=P)

        # gpsimd: constant for the matmul reduction
        nc.gpsimd.memset(ones[:], 1.0).then_inc(gsem, 1)

        # sync queue: chunks 0, 2; later the single-descriptor output DMA
        nc.sync.dma_start(t[:, CUTS[0] : CUTS[1]], chunk_src(0)).then_inc(c0sem, 16)
        nc.sync.dma_start(t[:, CUTS[2] : CUTS[3]], chunk_src(2)).then_inc(c2sem, 16)

        # scalar queue: chunks 1, 3
        nc.scalar.dma_start(t[:, CUTS[1] : CUTS[2]], chunk_src(1)).then_inc(c1sem, 16)
        nc.scalar.dma_start(t[:, CUTS[3] : CUTS[4]], chunk_src(3)).then_inc(c3sem, 16)

        # vector: per-chunk abs-sum partials, then PSUM -> SBUF copy
        for j in range(4):
            nc.vector.wait_ge(csems[j], 16)
            ins = nc.vector.tensor_reduce(
                pad[:, j : j + 1],
                t[:, CUTS[j] : CUTS[j + 1]],
                axis=mybir.AxisListType.X,
                op=mybir.AluOpType.add,
                apply_absolute_value=True,
            )
        ins.then_inc(vsem, 1)
        nc.vector.wait_ge(psem, 1)
        nc.vector.tensor_copy(zt[:], ps[0:1, 0:YCOLS]).then_inc(zsem, 1)

        # tensor: cross-partition reduction of the 4 partial columns
        nc.tensor.wait_ge(gsem, 1)
        nc.tensor.wait_ge(vsem, 1)
        nc.tensor.matmul(ps[0:1, 0:YCOLS], ones[:], pad[:, 0:YCOLS]).then_inc(psem, 1)

        # sync: output DMA once zt is ready
        nc.sync.wait_ge(zsem, 1)
        nc.sync.dma_start(y[:], zt[:]).then_inc(ysem, 16)
        nc.sync.wait_ge(ysem, 16)

    return nc


def _prep_in_maps(X: np.ndarray):
    X = np.ascontiguousarray(X, dtype=np.float32)
    starts = _starts()
    flats = []
    for c, (xs, ys) in enumerate(starts):
        flats.append(X[xs : xs + FRAME_S, ys : ys + FRAME_S].reshape(-1))

    in_maps = []
    for k in range(N_CORES):
        xk = np.zeros((P, F), dtype=np.float32)
        # class A = class k
        bufA = np.zeros(P * FA, dtype=np.float32)
        bufA[: FRAME_S * FRAME_S] = flats[k]
        xk[:, :FA] = bufA.reshape(P, FA)
        # class B = quarter (k%4) of class 8 + k//4
        cb = 8 + k // 4
        q = k % 4
        bufB = np.zeros(P * 4 * FB, dtype=np.float32)
        bufB[: FRAME_S * FRAME_S] = flats[cb]
        xk[:, FA:] = bufB.reshape(P, 4 * FB)[:, FB * q : FB * (q + 1)]
        # chunk-major flat layout
        flat = np.concatenate(
            [xk[:, CUTS[j] : CUTS[j + 1]].ravel() for j in range(4)]
        )
        in_maps.append({"x": flat})
    return in_maps


_NC = None


def kernel(X: np.ndarray) -> np.ndarray:
    global _NC
    if _NC is None:
        _NC = _build_bass()
    in_maps = _prep_in_maps(X)
    res = run_bass_kernel_spmd(_NC, in_maps, core_ids=list(range(N_CORES)))
    out = np.zeros(CLASSES, dtype=np.float32)
    for k in range(N_CORES):
        yk = res.results[k]["y"].reshape(-1)
        out[k] += yk[0:3].sum(dtype=np.float32)
        out[8 + k // 4] += yk[3]
    return out.astype(np.float32)
